# revision 1
# baseline (speedup 1.0000x reference)
"""HCLT probabilistic-circuit kernel for 8 Trainium2 NeuronCores.

Math: the reference collapses algebraically. With
  lp0 + lp1 summed in log space, exp'd, mixed by w_sum, then logsumexp'd,
the whole network is
  out[b] = log( sum_{k,m} w_sum[k] * W0[k,m,x0_b] * W1[k,m,x1_b] )
        = log( A[x0_b, x1_b] ),   A = sum_k w_k * W0[k].T @ W1[k]  (shape [C, C])

Distribution: shard the latent axis k (256) across 8 cores (32 each). Each core
reads only its W shard (134MB/8/2 in bf16 = 8.4MB), computes the partial
A_c = sum_{k in shard} w_k W0[k].T @ W1[k] via PSUM-accumulated matmuls, then
gathers its partial A_c at all 1024 (x0_b, x1_b) positions on-device
(one-hot matmul row-gather + fused mask-dot column-gather). The host sums the
8 partial gathered vectors (the unshard of the k-sharded reduction) and takes
the log.
"""

import sys

import numpy as np

sys.path.insert(0, "/opt/trn_rl_repo")

import ml_dtypes

B, V, M, C = 1024, 2, 256, 256
NCORES = 8
KSH = M // NCORES          # k per core = 32
KM = KSH * M               # flattened contraction rows per core = 8192
NCHUNK = KM // 128         # 64 matmul chunks of 128 rows
NBT = B // 128             # 8 batch tiles

_cache = {}


def _build_program():
    import concourse.bacc as bacc
    import concourse.mybir as mybir
    from concourse.tile import TileContext

    f32 = mybir.dt.float32
    bf16 = mybir.dt.bfloat16

    nc = bacc.Bacc("TRN2", target_bir_lowering=False)

    x0w = nc.dram_tensor("x0w", [128, NCHUNK * C], bf16, kind="ExternalInput")
    x1w = nc.dram_tensor("x1w", [128, NCHUNK * C], bf16, kind="ExternalInput")
    # aux packs f32 [wsc (64) | iota (256) | x1t (8)] per partition
    aux = nc.dram_tensor("aux", [128, NCHUNK + C + NBT], f32, kind="ExternalInput")
    oh0t = nc.dram_tensor("oh0t", [2 * 128, B], bf16, kind="ExternalInput")
    gout = nc.dram_tensor("gout", [128, NBT], f32, kind="ExternalOutput")

    NPIECE = 8
    PW = NCHUNK * C // NPIECE  # 2048 columns per DMA piece

    with TileContext(nc) as tc:
        with (
            tc.tile_pool(name="wp", bufs=1) as wp,
            tc.tile_pool(name="sp", bufs=3) as sp,
            tc.tile_pool(name="rp", bufs=4, space="PSUM") as rp,
            tc.tile_pool(name="apool", bufs=1, space="PSUM") as apool,
        ):
            x0sb = wp.tile([128, NCHUNK * C], bf16, name="x0sb")
            x1sb = wp.tile([128, NCHUNK * C], bf16, name="x1sb")
            x0s = wp.tile([128, NCHUNK * C], bf16, name="x0s")
            auxsb = wp.tile([128, NCHUNK + C + NBT], f32, name="auxsb")
            oh0sb = wp.tile([128, 2 * B], bf16, name="oh0sb")
            oh1sb = wp.tile([128, NBT * C], f32, name="oh1sb")
            gsb = wp.tile([128, NBT], f32, name="gsb")

            nc.sync.dma_start(out=auxsb[:], in_=aux[:])
            wscsb = auxsb[:, 0:NCHUNK]
            iotasb = auxsb[:, NCHUNK : NCHUNK + C]
            x1tsb = auxsb[:, NCHUNK + C : NCHUNK + C + NBT]

            # interleave the W-shard pieces so compute can chase the DMAs
            for p in range(NPIECE):
                sl = slice(p * PW, (p + 1) * PW)
                nc.sync.dma_start(out=x0sb[:, sl], in_=x0w[:, sl])
                nc.sync.dma_start(out=x1sb[:, sl], in_=x1w[:, sl])
            nc.sync.dma_start(out=oh0sb[:, 0:B], in_=oh0t[0:128, :])
            nc.sync.dma_start(out=oh0sb[:, B : 2 * B], in_=oh0t[128:256, :])

            # scale W0 chunks by their (uniform-per-chunk) w_sum factor
            for j in range(NCHUNK):
                sl = slice(j * C, (j + 1) * C)
                nc.vector.tensor_scalar(
                    out=x0s[:, sl],
                    in0=x0sb[:, sl],
                    scalar1=wscsb[:, j : j + 1],
                    scalar2=None,
                    op0=mybir.AluOpType.mult,
                )

            # build the 8 per-batch-tile x1 one-hot masks (needed only at
            # the gather stage; placed after the scales so the first scale
            # op -- which gates the first matmul -- issues as early as
            # possible on the in-order DVE queue)
            for i in range(NBT):
                nc.vector.tensor_scalar(
                    out=oh1sb[:, i * C : (i + 1) * C],
                    in0=iotasb,
                    scalar1=x1tsb[:, i : i + 1],
                    scalar2=None,
                    op0=mybir.AluOpType.is_equal,
                )

            # partial A = sum over 64 chunks of x0s_chunk.T @ x1_chunk
            a_ps = []
            for h in range(2):
                ah = apool.tile([128, C], f32, name=f"a{h}")
                a_ps.append(ah)
            # per DMA piece, run each PSUM half as a contiguous 8-MM
            # burst so LDWEIGHTS overlaps within a same-bank run
            CPP = NCHUNK // NPIECE
            for p in range(NPIECE):
                for h in range(2):
                    for j in range(p * CPP, (p + 1) * CPP):
                        nc.tensor.matmul(
                            a_ps[h],
                            lhsT=x0s[:, j * C + h * 128 : j * C + h * 128 + 128],
                            rhs=x1sb[:, j * C : (j + 1) * C],
                            start=(j == 0),
                            stop=(j == NCHUNK - 1),
                        )

            a_sb = []
            for h in range(2):
                ash = wp.tile([128, C], bf16, name=f"ash{h}")
                nc.vector.tensor_copy(ash, a_ps[h])
                a_sb.append(ash)

            # gather: R[b,:] = A[x0_b,:] via one-hot matmul, then dot with
            # the x1 one-hot row mask (built on-device) and reduce.
            for i in range(NBT):
                r_ps = rp.tile([128, C], mybir.dt.float32, name="r_ps")
                nc.tensor.matmul(
                    r_ps,
                    lhsT=oh0sb[:, i * 128 : (i + 1) * 128],
                    rhs=a_sb[0],
                    start=True,
                    stop=False,
                )
                nc.tensor.matmul(
                    r_ps,
                    lhsT=oh0sb[:, B + i * 128 : B + (i + 1) * 128],
                    rhs=a_sb[1],
                    start=False,
                    stop=True,
                )
                masked = sp.tile([128, C], f32, name="masked")
                nc.vector.tensor_tensor(
                    out=masked,
                    in0=r_ps,
                    in1=oh1sb[:, i * C : (i + 1) * C],
                    op=mybir.AluOpType.mult,
                )
                nc.vector.tensor_reduce(
                    out=gsb[:, i : i + 1],
                    in_=masked,
                    axis=mybir.AxisListType.X,
                    op=mybir.AluOpType.add,
                )

            nc.sync.dma_start(out=gout[:], in_=gsb[:])

    nc.compile()
    return nc


def _prep_inputs(x, W, w_sum):
    bf16 = ml_dtypes.bfloat16
    x = np.asarray(x)
    W = np.asarray(W, dtype=np.float32)
    w_sum = np.asarray(w_sum, dtype=np.float32)

    oh0t = np.zeros((C, B), dtype=bf16)
    oh0t[x[:, 0].astype(np.int64), np.arange(B)] = 1
    iotaf = np.broadcast_to(np.arange(C, dtype=np.float32)[None, :], (128, C))
    x1t = x[:, 1].astype(np.float32).reshape(NBT, 128).T

    in_maps = []
    for c in range(NCORES):
        k0 = c * KSH
        w0 = W[0, k0 : k0 + KSH].reshape(KM, C).astype(bf16)
        w1 = W[1, k0 : k0 + KSH].reshape(KM, C).astype(bf16)
        x0wc = np.ascontiguousarray(
            w0.reshape(NCHUNK, 128, C).transpose(1, 0, 2).reshape(128, NCHUNK * C)
        )
        x1wc = np.ascontiguousarray(
            w1.reshape(NCHUNK, 128, C).transpose(1, 0, 2).reshape(128, NCHUNK * C)
        )
        wsc = np.broadcast_to(
            np.repeat(w_sum[k0 : k0 + KSH], M // 128)[None, :], (128, NCHUNK)
        )
        auxc = np.ascontiguousarray(
            np.concatenate([wsc, iotaf, x1t], axis=1).astype(np.float32)
        )
        in_maps.append({"x0w": x0wc, "x1w": x1wc, "aux": auxc, "oh0t": oh0t})
    return in_maps


def _run(in_maps, **kwargs):
    from concourse.bass_utils import run_bass_kernel_spmd

    if "nc" not in _cache:
        _cache["nc"] = _build_program()
    return run_bass_kernel_spmd(
        _cache["nc"], in_maps, core_ids=list(range(NCORES)), **kwargs
    )


def kernel(x, W, w_sum):
    in_maps = _prep_inputs(x, W, w_sum)
    res = _run(in_maps)
    g = np.zeros((128, NBT), dtype=np.float64)
    for r in res.results:
        g += r["gout"].astype(np.float64)
    vals = g.T.reshape(B)  # b = tile*128 + partition
    return np.log(vals).astype(np.float32)



# revision 2
# speedup vs baseline: 1.5415x; 1.5415x over previous
"""HCLT probabilistic-circuit kernel for 8 Trainium2 NeuronCores.

Math: the reference collapses algebraically. With
  lp0 + lp1 summed in log space, exp'd, mixed by w_sum, then logsumexp'd,
the whole network is
  out[b] = log( sum_{k,m} w_sum[k] * W0[k,m,x0_b] * W1[k,m,x1_b] )
        = log( A[x0_b, x1_b] ),   A = sum_k w_k * W0[k].T @ W1[k]  (shape [C, C])

Distribution: shard the latent axis k (256) across 8 cores (32 each). Each
core reads only its W shard, quantized to fp8e4m3 on host (w_sum and a
power-of-two range scale folded in), computes its partial
A_c = sum_{km} w0q[km,:]^T w1q[km,:] with DoubleRow fp8 matmuls (two
128-row chunks contracted per instruction), and DMAs the [256,256] f32
partial back. The host sums the 8 partials (undoing each core's scale)
and evaluates log A at the 1024 (x0_b, x1_b) index pairs.

The kernel is HBM-bound: 2 x 2.1 MB of fp8 weights per core at the
~356 GB/s per-core DMA roofline ~= 11.8 us; the 64 DoubleRow matmuls
(6.8 us) hide under the DMA by chasing the weight pieces.
"""

import sys

import numpy as np

sys.path.insert(0, "/opt/trn_rl_repo")

import ml_dtypes

B, V, M, C = 1024, 2, 256, 256
NCORES = 8
KSH = M // NCORES          # k per core = 32
KM = KSH * M               # flattened contraction rows per core = 8192
NCHUNK = KM // 128         # 64 contraction chunks of 128 rows
NPAIR = NCHUNK // 2        # 32 DoubleRow chunk pairs
NPIECE = 8                 # DMA pieces per weight tensor
CHP = NCHUNK // NPIECE     # chunks per DMA piece = 8
PRP = NPAIR // NPIECE      # pairs per DMA piece = 4

_cache = {}


def _build_program():
    import concourse.bacc as bacc
    import concourse.mybir as mybir
    from concourse.tile import TileContext

    f32 = mybir.dt.float32
    fp8 = mybir.dt.float8e4

    nc = bacc.Bacc("TRN2", target_bir_lowering=False)

    x0w = nc.dram_tensor("x0w", [128, NCHUNK * C], fp8, kind="ExternalInput")
    x1w = nc.dram_tensor("x1w", [128, NCHUNK * C], fp8, kind="ExternalInput")
    gout = nc.dram_tensor("gout", [128, 2 * C], f32, kind="ExternalOutput")

    with TileContext(nc) as tc:
        with (
            tc.tile_pool(name="wp", bufs=1) as wp,
            tc.tile_pool(name="apool", bufs=1, space="PSUM") as apool,
        ):
            x0sb = wp.tile([128, NCHUNK, C], fp8, name="x0sb")
            x1sb = wp.tile([128, NCHUNK, C], fp8, name="x1sb")
            gsb = wp.tile([128, 2 * C], f32, name="gsb")

            # stream both weight shards in pieces so matmuls can chase
            for p in range(NPIECE):
                cs = slice(p * CHP, (p + 1) * CHP)
                fs = slice(p * CHP * C, (p + 1) * CHP * C)
                nc.sync.dma_start(out=x0sb[:, cs, :], in_=x0w[:, fs])
                nc.sync.dma_start(out=x1sb[:, cs, :], in_=x1w[:, fs])

            a_ps = [apool.tile([128, C], f32, name=f"a{h}") for h in range(2)]

            # partial A via DoubleRow fp8 matmuls: pair i contracts chunks
            # 2i,2i+1 (256 rows) in one instruction. Per piece, each PSUM
            # half runs as a contiguous burst so LDWEIGHTS pipelines.
            for p in range(NPIECE):
                for h in range(2):
                    for i in range(p * PRP, (p + 1) * PRP):
                        nc.tensor.matmul(
                            a_ps[h],
                            lhsT=x0sb[:, 2 * i : 2 * i + 2, h * 128 : h * 128 + 128],
                            rhs=x1sb[:, 2 * i : 2 * i + 2, :],
                            start=(i == 0),
                            stop=(i == NPAIR - 1),
                            perf_mode=mybir.MatmulPerfMode.DoubleRow,
                        )
                    if p == NPIECE - 1:
                        # drain this half while the other's burst still runs
                        nc.vector.tensor_copy(gsb[:, h * C : (h + 1) * C], a_ps[h])
                        nc.sync.dma_start(
                            out=gout[:, h * C : (h + 1) * C],
                            in_=gsb[:, h * C : (h + 1) * C],
                        )

    nc.compile()
    return nc


def _prep_inputs(x, W, w_sum):
    fp8 = ml_dtypes.float8_e4m3
    x = np.asarray(x)
    W = np.asarray(W, dtype=np.float32)
    w_sum = np.asarray(w_sum, dtype=np.float32)

    in_maps = []
    scales = []
    for c in range(NCORES):
        k0 = c * KSH
        w0 = (W[0, k0 : k0 + KSH] * w_sum[k0 : k0 + KSH, None, None]).reshape(KM, C)
        w1 = W[1, k0 : k0 + KSH].reshape(KM, C)
        # power-of-two scales put each shard's max near 128 (safe for any
        # e4m3 flavor) without adding rounding error of their own
        s0 = 2.0 ** np.floor(np.log2(128.0 / w0.max()))
        s1 = 2.0 ** np.floor(np.log2(128.0 / w1.max()))
        # chunk-major [128, NCHUNK*C]: partition p = row within chunk
        x0wc = np.ascontiguousarray(
            (w0 * s0).astype(fp8).reshape(NCHUNK, 128, C).transpose(1, 0, 2).reshape(128, NCHUNK * C)
        )
        x1wc = np.ascontiguousarray(
            (w1 * s1).astype(fp8).reshape(NCHUNK, 128, C).transpose(1, 0, 2).reshape(128, NCHUNK * C)
        )
        in_maps.append({"x0w": x0wc, "x1w": x1wc})
        scales.append(1.0 / (float(s0) * float(s1)))
    return in_maps, scales


def _run(in_maps, **kwargs):
    from concourse.bass_utils import run_bass_kernel_spmd

    if "nc" not in _cache:
        _cache["nc"] = _build_program()
    return run_bass_kernel_spmd(
        _cache["nc"], in_maps, core_ids=list(range(NCORES)), **kwargs
    )


def _unshard(results, scales, x):
    x = np.asarray(x)
    A = np.zeros((C, C), dtype=np.float64)
    for r, inv_s in zip(results, scales):
        # gout[p, h*C + c] = A_c[h*128 + p, c]
        Ac = r["gout"].reshape(128, 2, C).transpose(1, 0, 2).reshape(C, C)
        A += Ac.astype(np.float64) * inv_s
    vals = A[x[:, 0].astype(np.int64), x[:, 1].astype(np.int64)]
    return np.log(vals).astype(np.float32)


def kernel(x, W, w_sum):
    in_maps, scales = _prep_inputs(x, W, w_sum)
    res = _run(in_maps)
    return _unshard(res.results, scales, x)
